# revision 6
# baseline (speedup 1.0000x reference)
"""Trainium2 Bass kernel for nn_CriticEstimator.

Math (per batch row b, agent n):
    x    = [state, action]                      # [192]
    e    = x @ G_n + gb_n                       # [256]
    v    = relu(e @ W_V^T)                      # [256]
    (the reference's attention einsum 'bhnm,bhnd->bhnd' multiplies v by
     softmax row-sums == 1, so the attention block is an exact pass-through)
    h    = relu([e, v] @ W1_n + b1_n)           # [256]
    q    = h @ w2_n + b2_n                      # [1]

Since e has no nonlinearity, fold it into the weights (host-side, fp64):
    A_n = G_n @ W_V^T            v = relu(x @ A_n + c_v),  c_v = gb_n @ W_V^T
    P_n = G_n @ U1_n             h = relu(x @ P_n + v @ U2_n + c_h)
                                 c_h = gb_n @ U1_n + b1_n
    (U1_n = W1_n[:256], U2_n = W1_n[256:])
b2 is added on the host after gathering.

Device layout: feature-major activations (x^T shipped from host), so every
matmul contracts over SBUF partitions with zero on-chip transposes.  Agents
are sharded 2-per-core across 8 cores; each core streams the full batch in
512-column tiles.  Operands are bf16 (fp32 PSUM accumulate; rel err ~4e-3):
halves DMA traffic vs fp32r and streams K=64 chunks at full rate, so the
action chunks need no zero-padding.  14 matmul passes per (agent, tile) is
pass-optimal for this decomposition (v: 2Kx2M, h: 4Kx2M, q: 2).  Biases are
applied during PSUM eviction (ACT relu+bias for v, DVE tensor_scalar for h);
q is written straight from PSUM partition 0 to DRAM per tile, removing the
DVE copy and the serial single-partition output DMA of the fp32r version.
"""

import sys

if "/opt/trn_rl_repo" not in sys.path:
    sys.path.insert(0, "/opt/trn_rl_repo")

import numpy as np
import ml_dtypes

import concourse.bass as bass
import concourse.mybir as mybir
from concourse import bacc
from concourse.tile import TileContext
from concourse.bass_utils import run_bass_kernel_spmd

B, N, S, A, E = 8192, 16, 128, 64, 256
IN = S + A                     # 192
NCORES = 8
G = N // NCORES                # agents per core
BT = 512                       # batch columns per tile (one PSUM bank)
NBT = B // BT
K0 = 128                       # x-feature chunk 0: features 0..127 (state)
F32 = mybir.dt.float32
BF16 = mybir.dt.bfloat16
NPBF = ml_dtypes.bfloat16
RELU = mybir.ActivationFunctionType.Relu
COPY = mybir.ActivationFunctionType.Copy
ADD = mybir.AluOpType.add
MAX = mybir.AluOpType.max

_BUILT = {}
ABLATE = ""


def _build(repeats=1):
    key = (repeats, ABLATE)
    if key in _BUILT:
        return _BUILT[key]

    nc = bacc.Bacc("TRN2", target_bir_lowering=False, debug=False,
                   num_devices=NCORES)

    xk0 = nc.dram_tensor("xk0", [G, K0, B], BF16, kind="ExternalInput").ap()
    xk1 = nc.dram_tensor("xk1", [G, A, B], BF16, kind="ExternalInput").ap()
    wa = nc.dram_tensor("wa", [G, K0, E], BF16, kind="ExternalInput").ap()
    wp = nc.dram_tensor("wp", [G, K0, E], BF16, kind="ExternalInput").ap()
    wa1 = nc.dram_tensor("wa1", [G, A, E], BF16, kind="ExternalInput").ap()
    wp1 = nc.dram_tensor("wp1", [G, A, E], BF16, kind="ExternalInput").ap()
    wu = nc.dram_tensor("wu", [G, E, E], BF16, kind="ExternalInput").ap()
    # w2 chunks replicated across 128 output columns (tiny-M matmuls run
    # below full rate on hw; M=128 with identical columns streams full rate)
    w2 = nc.dram_tensor("w2", [G, 128, 2 * 128], BF16, kind="ExternalInput").ap()
    cv = nc.dram_tensor("cv", [G, 128, 2], F32, kind="ExternalInput").ap()
    ch = nc.dram_tensor("ch", [G, 128, 2], F32, kind="ExternalInput").ap()
    y = nc.dram_tensor("y", [G, B], F32, kind="ExternalOutput").ap()

    with TileContext(nc) as tc:
        with (
            tc.tile_pool(name="wpool", bufs=2) as wpool,
            tc.tile_pool(name="xpool", bufs=8) as xpool,
            tc.tile_pool(name="vpool", bufs=4) as vpool,
            tc.tile_pool(name="hpool", bufs=4) as hpool,
            tc.tile_pool(name="qpool", bufs=4) as qpool,
            tc.tile_pool(name="pv", bufs=2, space="PSUM") as pvpool,
            tc.tile_pool(name="ph", bufs=4, space="PSUM") as phpool,
            tc.tile_pool(name="pq", bufs=2, space="PSUM") as pqpool,
            _repeat_loop(tc, repeats),
        ):
            for g in range(G):
                wa_t = wpool.tile([K0, E], BF16)
                nc.sync.dma_start(out=wa_t[:], in_=wa[g, :, :])
                wp_t = wpool.tile([K0, E], BF16)
                nc.sync.dma_start(out=wp_t[:], in_=wp[g, :, :])
                wa1_t = wpool.tile([A, E], BF16)
                nc.sync.dma_start(out=wa1_t[:], in_=wa1[g, :, :])
                wp1_t = wpool.tile([A, E], BF16)
                nc.sync.dma_start(out=wp1_t[:], in_=wp1[g, :, :])
                wu0_t = wpool.tile([128, E], BF16)
                nc.sync.dma_start(out=wu0_t[:], in_=wu[g, :128, :])
                wu1_t = wpool.tile([128, E], BF16)
                nc.sync.dma_start(out=wu1_t[:], in_=wu[g, 128:, :])
                w2_t = wpool.tile([128, 2 * 128], BF16)
                nc.sync.dma_start(out=w2_t[:], in_=w2[g, :, :])
                cv_t = wpool.tile([128, 2], F32)
                nc.sync.dma_start(out=cv_t[:], in_=cv[g, :, :])
                ch_t = wpool.tile([128, 2], F32)
                nc.sync.dma_start(out=ch_t[:], in_=ch[g, :, :])

                def emit_q(hts_prev, bt_prev):
                    c0p, c1p = bt_prev * BT, (bt_prev + 1) * BT
                    pq = pqpool.tile([128, BT], F32)
                    nc.tensor.matmul(pq[:], w2_t[:, 0:128], hts_prev[0][:],
                                     start=True, stop=False)
                    nc.tensor.matmul(pq[:], w2_t[:, 128:256], hts_prev[1][:],
                                     start=False, stop=True)
                    if ABLATE != "mm_only":
                        # q lives on PSUM partition 0 (w2 replicated over M);
                        # ACT copies it out (Pool can't read PSUM on hw, and
                        # DVE is busy with h evictions; copy shares the relu
                        # act table so no table reload)
                        qt = qpool.tile([1, BT], F32)
                        nc.scalar.activation(qt[:], pq[0:1, :], COPY)
                        nc.sync.dma_start(out=y[g, c0p:c1p], in_=qt[:])

                prev_hts = None
                for bt in range(NBT):
                    c0, c1 = bt * BT, (bt + 1) * BT
                    xt0 = xpool.tile([K0, BT], BF16)
                    nc.sync.dma_start(out=xt0[:], in_=xk0[g, :, c0:c1])
                    xt1 = xpool.tile([A, BT], BF16)
                    nc.sync.dma_start(out=xt1[:], in_=xk1[g, :, c0:c1])

                    vts = []
                    for mh in range(2):
                        ms = slice(mh * 128, (mh + 1) * 128)
                        pv = pvpool.tile([128, BT], F32)
                        nc.tensor.matmul(pv[:], wa_t[:, ms], xt0[:],
                                         start=True, stop=False)
                        nc.tensor.matmul(pv[:], wa1_t[:, ms], xt1[:],
                                         start=False, stop=True)
                        if ABLATE != "mm_only":
                            vt = vpool.tile([128, BT], BF16)
                            nc.scalar.activation(vt[:], pv[:], RELU,
                                                 bias=cv_t[:, mh:mh + 1])
                            vts.append(vt)

                    # previous tile's q-stage fills PE time while this tile's
                    # evictions drain
                    if prev_hts is not None:
                        emit_q(prev_hts, bt - 1)

                    rhs0 = vts[0] if ABLATE != "mm_only" else xt0
                    rhs1 = vts[1] if ABLATE != "mm_only" else xt0
                    hts = []
                    for mh in range(2):
                        ms = slice(mh * 128, (mh + 1) * 128)
                        ph = phpool.tile([128, BT], F32)
                        nc.tensor.matmul(ph[:], wp1_t[:, ms], xt1[:],
                                         start=True, stop=False)
                        nc.tensor.matmul(ph[:], wp_t[:, ms], xt0[:],
                                         start=False, stop=False)
                        nc.tensor.matmul(ph[:], wu0_t[:, ms], rhs0[:],
                                         start=False, stop=False)
                        nc.tensor.matmul(ph[:], wu1_t[:, ms], rhs1[:],
                                         start=False, stop=True)
                        if ABLATE != "mm_only":
                            ht = hpool.tile([128, BT], BF16)
                            nc.vector.tensor_scalar(ht[:], ph[:],
                                                    ch_t[:, mh:mh + 1], 0.0,
                                                    op0=ADD, op1=MAX)
                            hts.append(ht)
                    prev_hts = hts if ABLATE != "mm_only" else [xt0, xt0]

                if prev_hts is not None:
                    emit_q(prev_hts, NBT - 1)

    nc.finalize()
    _BUILT[key] = nc
    return nc


def _repeat_loop(tc, repeats):
    # benchmarking aid: run the whole body `repeats` times so wall-clock
    # differences isolate on-device execution time
    from contextlib import nullcontext
    return tc.For_i(0, repeats, 1) if repeats > 1 else nullcontext()


def _prep_in_maps(state, action, g_weight, g_bias, W_Q, W_K, W_V,
                  gx_weight_1, gx_bias_1, gx_weight_2, gx_bias_2):
    f8 = np.float64
    WVt = W_V.T.astype(f8)
    in_maps = []
    for c in range(NCORES):
        xk0 = np.empty((G, K0, B), NPBF)
        xk1 = np.empty((G, A, B), NPBF)
        wa = np.empty((G, K0, E), NPBF)
        wp = np.empty((G, K0, E), NPBF)
        wa1 = np.empty((G, A, E), NPBF)
        wp1 = np.empty((G, A, E), NPBF)
        wu = np.empty((G, E, E), NPBF)
        w2 = np.empty((G, 128, 2 * 128), NPBF)
        cv = np.empty((G, 128, 2), np.float32)
        ch = np.empty((G, 128, 2), np.float32)
        for g in range(G):
            n = c * G + g
            xk0[g] = state[:, n, :].T
            xk1[g] = action[:, n, :].T
            Gn = g_weight[n].astype(f8)
            U1 = gx_weight_1[n][:E].astype(f8)
            An = (Gn @ WVt).astype(np.float32)
            Pn = (Gn @ U1).astype(np.float32)
            wa[g] = An[:K0]
            wp[g] = Pn[:K0]
            wa1[g] = An[K0:]
            wp1[g] = Pn[K0:]
            wu[g] = gx_weight_1[n][E:]
            w2[g, :, 0:128] = gx_weight_2[n][:128, 0:1]
            w2[g, :, 128:256] = gx_weight_2[n][128:, 0:1]
            cvn = (g_bias[n].astype(f8) @ WVt).astype(np.float32)
            chn = (g_bias[n].astype(f8) @ U1
                   + gx_bias_1[n].astype(f8)).astype(np.float32)
            cv[g, :, 0] = cvn[:128]
            cv[g, :, 1] = cvn[128:]
            ch[g, :, 0] = chn[:128]
            ch[g, :, 1] = chn[128:]
        in_maps.append({
            "xk0": np.ascontiguousarray(xk0),
            "xk1": np.ascontiguousarray(xk1),
            "wa": np.ascontiguousarray(wa),
            "wp": np.ascontiguousarray(wp),
            "wa1": np.ascontiguousarray(wa1),
            "wp1": np.ascontiguousarray(wp1),
            "wu": np.ascontiguousarray(wu),
            "w2": np.ascontiguousarray(w2),
            "cv": np.ascontiguousarray(cv),
            "ch": np.ascontiguousarray(ch),
        })
    return in_maps


def _run(in_maps, repeats=1, **kwargs):
    nc = _build(repeats)
    return run_bass_kernel_spmd(nc, in_maps, list(range(NCORES)), **kwargs)


def _gather(results, gx_bias_2):
    out = np.empty((B, N, 1), np.float32)
    for c in range(NCORES):
        yc = results[c]["y"]
        for g in range(G):
            n = c * G + g
            out[:, n, 0] = yc[g] + gx_bias_2[n, 0]
    return out


def kernel(**inputs):
    inputs = {k: np.asarray(v) for k, v in inputs.items()}
    in_maps = _prep_in_maps(**inputs)
    res = _run(in_maps)
    return _gather(res.results, inputs["gx_bias_2"])


# revision 9
# speedup vs baseline: 1.2043x; 1.2043x over previous
"""Trainium2 Bass kernel for nn_CriticEstimator.

Math (per batch row b, agent n):
    x    = [state, action]                      # [192]
    e    = x @ G_n + gb_n                       # [256]
    v    = relu(e @ W_V^T)                      # [256]
    (the reference's attention einsum 'bhnm,bhnd->bhnd' multiplies v by
     softmax row-sums == 1, so the attention block is an exact pass-through)
    h    = relu([e, v] @ W1_n + b1_n)           # [256]
    q    = h @ w2_n + b2_n                      # [1]

Since e has no nonlinearity, fold it into the weights (host-side, fp64):
    A_n = G_n @ W_V^T            v = relu(x @ A_n + c_v),  c_v = gb_n @ W_V^T
    P_n = G_n @ U1_n             h = relu(x @ P_n + v @ U2_n + c_h)
                                 c_h = gb_n @ U1_n + b1_n
    (U1_n = W1_n[:256], U2_n = W1_n[256:])
b2 is added on the host after gathering.

Device layout: feature-major activations (x^T shipped from host), so every
matmul contracts over SBUF partitions with zero on-chip transposes.  Agents
are sharded 2-per-core across 8 cores; each core streams the full batch in
512-column tiles.  Operands are bf16 (fp32 PSUM accumulate; rel err ~4e-3):
halves DMA traffic vs fp32r and streams K=64 chunks at full rate, so the
action chunks need no zero-padding.  14 matmul passes per (agent, tile) is
pass-optimal for this decomposition (v: 2Kx2M, h: 4Kx2M, q: 2).  Biases are
applied during PSUM eviction (ACT relu+bias for v, DVE tensor_scalar for h);
q is written straight from PSUM partition 0 to DRAM per tile, removing the
DVE copy and the serial single-partition output DMA of the fp32r version.
"""

import sys

if "/opt/trn_rl_repo" not in sys.path:
    sys.path.insert(0, "/opt/trn_rl_repo")

import numpy as np
import ml_dtypes

import concourse.bass as bass
import concourse.mybir as mybir
from concourse import bacc
from concourse.tile import TileContext
from concourse.bass_utils import run_bass_kernel_spmd

B, N, S, A, E = 8192, 16, 128, 64, 256
IN = S + A                     # 192
NCORES = 8
G = N // NCORES                # agents per core
BT = 512                       # batch columns per tile (one PSUM bank)
NBT = B // BT
K0 = 128                       # x-feature chunk 0: features 0..127 (state)
F32 = mybir.dt.float32
BF16 = mybir.dt.bfloat16
NPBF = ml_dtypes.bfloat16
RELU = mybir.ActivationFunctionType.Relu
COPY = mybir.ActivationFunctionType.Copy
ADD = mybir.AluOpType.add
MAX = mybir.AluOpType.max

_BUILT = {}
ABLATE = ""


def _build(repeats=1):
    key = (repeats, ABLATE)
    if key in _BUILT:
        return _BUILT[key]

    nc = bacc.Bacc("TRN2", target_bir_lowering=False, debug=False,
                   num_devices=NCORES)

    xk0 = nc.dram_tensor("xk0", [G, K0, B], BF16, kind="ExternalInput").ap()
    xk1 = nc.dram_tensor("xk1", [G, A, B], BF16, kind="ExternalInput").ap()
    wa = nc.dram_tensor("wa", [G, K0, E], BF16, kind="ExternalInput").ap()
    wp = nc.dram_tensor("wp", [G, K0, E], BF16, kind="ExternalInput").ap()
    # action-chunk weights padded to K=128 with zero rows (64-127): K<128
    # matmuls run ~2x slower on hw (measured 464ns vs 238ns at K=128 bf16)
    wa1 = nc.dram_tensor("wa1", [G, 128, E], BF16, kind="ExternalInput").ap()
    wp1 = nc.dram_tensor("wp1", [G, 128, E], BF16, kind="ExternalInput").ap()
    wu = nc.dram_tensor("wu", [G, E, E], BF16, kind="ExternalInput").ap()
    # w2 chunks replicated across 128 output columns (tiny-M matmuls run
    # below full rate on hw; M=128 with identical columns streams full rate)
    w2 = nc.dram_tensor("w2", [G, 128, 2 * 128], BF16, kind="ExternalInput").ap()
    cv = nc.dram_tensor("cv", [G, 128, 2], F32, kind="ExternalInput").ap()
    ch = nc.dram_tensor("ch", [G, 128, 2], F32, kind="ExternalInput").ap()
    y = nc.dram_tensor("y", [G, B], F32, kind="ExternalOutput").ap()

    with TileContext(nc) as tc:
        with (
            tc.tile_pool(name="wpool", bufs=2) as wpool,
            tc.tile_pool(name="xpool", bufs=4) as xpool,
            tc.tile_pool(name="x1pool", bufs=4) as x1pool,
            tc.tile_pool(name="vpool", bufs=4) as vpool,
            tc.tile_pool(name="hpool", bufs=4) as hpool,
            tc.tile_pool(name="qpool", bufs=4) as qpool,
            tc.tile_pool(name="pv", bufs=2, space="PSUM") as pvpool,
            tc.tile_pool(name="ph", bufs=4, space="PSUM") as phpool,
            tc.tile_pool(name="pq", bufs=2, space="PSUM") as pqpool,
            _repeat_loop(tc, repeats),
        ):
            # four rotating xt1 tiles with persistent zero rows 64-127
            # (padded-K companions); DMA only ever rewrites rows 0-63
            xt1s = []
            for _ in range(4):
                t = x1pool.tile([128, BT], BF16)
                nc.gpsimd.memset(t[A:, :].bitcast(F32), 0.0)
                xt1s.append(t)
            for g in range(G):
                wa_t = wpool.tile([K0, E], BF16)
                nc.sync.dma_start(out=wa_t[:], in_=wa[g, :, :])
                wp_t = wpool.tile([K0, E], BF16)
                nc.sync.dma_start(out=wp_t[:], in_=wp[g, :, :])
                wa1_t = wpool.tile([128, E], BF16)
                nc.sync.dma_start(out=wa1_t[:], in_=wa1[g, :, :])
                wp1_t = wpool.tile([128, E], BF16)
                nc.sync.dma_start(out=wp1_t[:], in_=wp1[g, :, :])
                wu0_t = wpool.tile([128, E], BF16)
                nc.sync.dma_start(out=wu0_t[:], in_=wu[g, :128, :])
                wu1_t = wpool.tile([128, E], BF16)
                nc.sync.dma_start(out=wu1_t[:], in_=wu[g, 128:, :])
                w2_t = wpool.tile([128, 2 * 128], BF16)
                nc.sync.dma_start(out=w2_t[:], in_=w2[g, :, :])
                cv_t = wpool.tile([128, 2], F32)
                nc.sync.dma_start(out=cv_t[:], in_=cv[g, :, :])
                ch_t = wpool.tile([128, 2], F32)
                nc.sync.dma_start(out=ch_t[:], in_=ch[g, :, :])

                def emit_q(hts_prev, bt_prev):
                    c0p, c1p = bt_prev * BT, (bt_prev + 1) * BT
                    pq = pqpool.tile([128, BT], F32)
                    nc.tensor.matmul(pq[:], w2_t[:, 0:128], hts_prev[0][:],
                                     start=True, stop=False)
                    nc.tensor.matmul(pq[:], w2_t[:, 128:256], hts_prev[1][:],
                                     start=False, stop=True)
                    if ABLATE != "mm_only":
                        # q lives on PSUM partition 0 (w2 replicated over M);
                        # ACT copies it out (Pool can't read PSUM on hw, and
                        # DVE is busy with h evictions; copy shares the relu
                        # act table so no table reload)
                        qt = qpool.tile([1, BT], F32)
                        nc.scalar.activation(qt[:], pq[0:1, :], COPY)
                        nc.sync.dma_start(out=y[g, c0p:c1p], in_=qt[:])

                prev_hts = None
                for bt in range(NBT):
                    c0, c1 = bt * BT, (bt + 1) * BT
                    xt0 = xpool.tile([K0, BT], BF16)
                    nc.sync.dma_start(out=xt0[:], in_=xk0[g, :, c0:c1])
                    xt1 = xt1s[bt % 4]
                    nc.sync.dma_start(out=xt1[0:A, :], in_=xk1[g, :, c0:c1])

                    vts = []
                    for mh in range(2):
                        ms = slice(mh * 128, (mh + 1) * 128)
                        pv = pvpool.tile([128, BT], F32)
                        nc.tensor.matmul(pv[:], wa_t[:, ms], xt0[:],
                                         start=True, stop=False)
                        nc.tensor.matmul(pv[:], wa1_t[:, ms], xt1[:],
                                         start=False, stop=True)
                        if ABLATE != "mm_only":
                            vt = vpool.tile([128, BT], BF16)
                            nc.scalar.activation(vt[:], pv[:], RELU,
                                                 bias=cv_t[:, mh:mh + 1])
                            vts.append(vt)

                    # previous tile's q-stage fills PE time while this tile's
                    # evictions drain
                    if prev_hts is not None:
                        emit_q(prev_hts, bt - 1)

                    rhs0 = vts[0] if ABLATE != "mm_only" else xt0
                    rhs1 = vts[1] if ABLATE != "mm_only" else xt0
                    hts = []
                    for mh in range(2):
                        ms = slice(mh * 128, (mh + 1) * 128)
                        ph = phpool.tile([128, BT], F32)
                        nc.tensor.matmul(ph[:], wp1_t[:, ms], xt1[:],
                                         start=True, stop=False)
                        nc.tensor.matmul(ph[:], wp_t[:, ms], xt0[:],
                                         start=False, stop=False)
                        nc.tensor.matmul(ph[:], wu0_t[:, ms], rhs0[:],
                                         start=False, stop=False)
                        nc.tensor.matmul(ph[:], wu1_t[:, ms], rhs1[:],
                                         start=False, stop=True)
                        if ABLATE != "mm_only":
                            ht = hpool.tile([128, BT], BF16)
                            nc.vector.tensor_scalar(ht[:], ph[:],
                                                    ch_t[:, mh:mh + 1], 0.0,
                                                    op0=ADD, op1=MAX)
                            hts.append(ht)
                    prev_hts = hts if ABLATE != "mm_only" else [xt0, xt0]

                if prev_hts is not None:
                    emit_q(prev_hts, NBT - 1)

    nc.finalize()
    _BUILT[key] = nc
    return nc


def _repeat_loop(tc, repeats):
    # benchmarking aid: run the whole body `repeats` times so wall-clock
    # differences isolate on-device execution time
    from contextlib import nullcontext
    return tc.For_i(0, repeats, 1) if repeats > 1 else nullcontext()


def _prep_in_maps(state, action, g_weight, g_bias, W_Q, W_K, W_V,
                  gx_weight_1, gx_bias_1, gx_weight_2, gx_bias_2):
    f8 = np.float64
    WVt = W_V.T.astype(f8)
    in_maps = []
    for c in range(NCORES):
        xk0 = np.empty((G, K0, B), NPBF)
        xk1 = np.empty((G, A, B), NPBF)
        wa = np.empty((G, K0, E), NPBF)
        wp = np.empty((G, K0, E), NPBF)
        wa1 = np.zeros((G, 128, E), NPBF)
        wp1 = np.zeros((G, 128, E), NPBF)
        wu = np.empty((G, E, E), NPBF)
        w2 = np.empty((G, 128, 2 * 128), NPBF)
        cv = np.empty((G, 128, 2), np.float32)
        ch = np.empty((G, 128, 2), np.float32)
        for g in range(G):
            n = c * G + g
            xk0[g] = state[:, n, :].T
            xk1[g] = action[:, n, :].T
            Gn = g_weight[n].astype(f8)
            U1 = gx_weight_1[n][:E].astype(f8)
            An = (Gn @ WVt).astype(np.float32)
            Pn = (Gn @ U1).astype(np.float32)
            wa[g] = An[:K0]
            wp[g] = Pn[:K0]
            wa1[g, :A] = An[K0:]
            wp1[g, :A] = Pn[K0:]
            wu[g] = gx_weight_1[n][E:]
            w2[g, :, 0:128] = gx_weight_2[n][:128, 0:1]
            w2[g, :, 128:256] = gx_weight_2[n][128:, 0:1]
            cvn = (g_bias[n].astype(f8) @ WVt).astype(np.float32)
            chn = (g_bias[n].astype(f8) @ U1
                   + gx_bias_1[n].astype(f8)).astype(np.float32)
            cv[g, :, 0] = cvn[:128]
            cv[g, :, 1] = cvn[128:]
            ch[g, :, 0] = chn[:128]
            ch[g, :, 1] = chn[128:]
        in_maps.append({
            "xk0": np.ascontiguousarray(xk0),
            "xk1": np.ascontiguousarray(xk1),
            "wa": np.ascontiguousarray(wa),
            "wp": np.ascontiguousarray(wp),
            "wa1": np.ascontiguousarray(wa1),
            "wp1": np.ascontiguousarray(wp1),
            "wu": np.ascontiguousarray(wu),
            "w2": np.ascontiguousarray(w2),
            "cv": np.ascontiguousarray(cv),
            "ch": np.ascontiguousarray(ch),
        })
    return in_maps


def _run(in_maps, repeats=1, **kwargs):
    nc = _build(repeats)
    return run_bass_kernel_spmd(nc, in_maps, list(range(NCORES)), **kwargs)


def _gather(results, gx_bias_2):
    out = np.empty((B, N, 1), np.float32)
    for c in range(NCORES):
        yc = results[c]["y"]
        for g in range(G):
            n = c * G + g
            out[:, n, 0] = yc[g] + gx_bias_2[n, 0]
    return out


def kernel(**inputs):
    inputs = {k: np.asarray(v) for k, v in inputs.items()}
    in_maps = _prep_in_maps(**inputs)
    res = _run(in_maps)
    return _gather(res.results, inputs["gx_bias_2"])


# revision 10
# speedup vs baseline: 1.2595x; 1.0458x over previous
"""Trainium2 Bass kernel for nn_CriticEstimator.

Math (per batch row b, agent n):
    x    = [state, action]                      # [192]
    e    = x @ G_n + gb_n                       # [256]
    v    = relu(e @ W_V^T)                      # [256]
    (the reference's attention einsum 'bhnm,bhnd->bhnd' multiplies v by
     softmax row-sums == 1, so the attention block is an exact pass-through)
    h    = relu([e, v] @ W1_n + b1_n)           # [256]
    q    = h @ w2_n + b2_n                      # [1]

Since e has no nonlinearity, fold it into the weights (host-side, fp64):
    A_n = G_n @ W_V^T            v = relu(x @ A_n + c_v),  c_v = gb_n @ W_V^T
    P_n = G_n @ U1_n             h = relu(x @ P_n + v @ U2_n + c_h)
                                 c_h = gb_n @ U1_n + b1_n
    (U1_n = W1_n[:256], U2_n = W1_n[256:])
b2 is added on the host after gathering.

Device layout: feature-major activations (x^T shipped from host), so every
matmul contracts over SBUF partitions with zero on-chip transposes.  Agents
are sharded 2-per-core across 8 cores; each core streams the full batch in
512-column tiles.  Operands are bf16 (fp32 PSUM accumulate; rel err ~4e-3):
measured 238ns per [128,128]x[128,512] matmul vs 280ns for fp32r, and DMA
bytes halve.  K<128 matmuls run ~2x slower on hw, so the K=64 action chunks
are zero-padded to 128 (weights host-side; x rows via one-time memsets into
four persistent xt1 tiles whose zero rows survive rotation).  14 matmul
passes per (agent, tile) is pass-optimal for this decomposition (v: 2Kx2M,
h: 4Kx2M, q: 2).  Biases are applied during PSUM eviction (ACT relu+bias
for v, DVE tensor_scalar for h); q is copied off PSUM partition 0 by ACT
(copy shares the relu act table; Pool cannot read PSUM) and DMA'd to DRAM
per tile, removing the serial single-partition output DMA at the end.
"""

import sys

if "/opt/trn_rl_repo" not in sys.path:
    sys.path.insert(0, "/opt/trn_rl_repo")

import numpy as np
import ml_dtypes

import concourse.bass as bass
import concourse.mybir as mybir
from concourse import bacc
from concourse.tile import TileContext
from concourse.bass_utils import run_bass_kernel_spmd

B, N, S, A, E = 8192, 16, 128, 64, 256
IN = S + A                     # 192
NCORES = 8
G = N // NCORES                # agents per core
BT = 512                       # batch columns per tile (one PSUM bank)
NBT = B // BT
K0 = 128                       # x-feature chunk 0: features 0..127 (state)
F32 = mybir.dt.float32
BF16 = mybir.dt.bfloat16
NPBF = ml_dtypes.bfloat16
RELU = mybir.ActivationFunctionType.Relu
COPY = mybir.ActivationFunctionType.Copy
ADD = mybir.AluOpType.add
MAX = mybir.AluOpType.max

_BUILT = {}
ABLATE = ""


def _build(repeats=1):
    key = (repeats, ABLATE)
    if key in _BUILT:
        return _BUILT[key]

    nc = bacc.Bacc("TRN2", target_bir_lowering=False, debug=False,
                   num_devices=NCORES)

    xk0 = nc.dram_tensor("xk0", [G, K0, B], BF16, kind="ExternalInput").ap()
    xk1 = nc.dram_tensor("xk1", [G, A, B], BF16, kind="ExternalInput").ap()
    wa = nc.dram_tensor("wa", [G, K0, E], BF16, kind="ExternalInput").ap()
    wp = nc.dram_tensor("wp", [G, K0, E], BF16, kind="ExternalInput").ap()
    # action-chunk weights padded to K=128 with zero rows (64-127): K<128
    # matmuls run ~2x slower on hw (measured 464ns vs 238ns at K=128 bf16)
    wa1 = nc.dram_tensor("wa1", [G, 128, E], BF16, kind="ExternalInput").ap()
    wp1 = nc.dram_tensor("wp1", [G, 128, E], BF16, kind="ExternalInput").ap()
    wu = nc.dram_tensor("wu", [G, E, E], BF16, kind="ExternalInput").ap()
    # w2 chunks replicated across 128 output columns (tiny-M matmuls run
    # below full rate on hw; M=128 with identical columns streams full rate)
    w2 = nc.dram_tensor("w2", [G, 128, 2 * 128], BF16, kind="ExternalInput").ap()
    cv = nc.dram_tensor("cv", [G, 128, 2], F32, kind="ExternalInput").ap()
    ch = nc.dram_tensor("ch", [G, 128, 2], F32, kind="ExternalInput").ap()
    y = nc.dram_tensor("y", [G, B], F32, kind="ExternalOutput").ap()

    with TileContext(nc) as tc:
        with (
            tc.tile_pool(name="wpool", bufs=2) as wpool,
            tc.tile_pool(name="xpool", bufs=4) as xpool,
            tc.tile_pool(name="x1pool", bufs=4) as x1pool,
            tc.tile_pool(name="vpool", bufs=4) as vpool,
            tc.tile_pool(name="hpool", bufs=4) as hpool,
            tc.tile_pool(name="qpool", bufs=4) as qpool,
            tc.tile_pool(name="pv", bufs=2, space="PSUM") as pvpool,
            tc.tile_pool(name="ph", bufs=4, space="PSUM") as phpool,
            tc.tile_pool(name="pq", bufs=2, space="PSUM") as pqpool,
            _repeat_loop(tc, repeats),
        ):
            # four rotating xt1 tiles with persistent zero rows 64-127
            # (padded-K companions); DMA only ever rewrites rows 0-63
            xt1s = []
            for _ in range(4):
                t = x1pool.tile([128, BT], BF16)
                nc.gpsimd.memset(t[A:, :].bitcast(F32), 0.0)
                xt1s.append(t)
            for g in range(G):
                wa_t = wpool.tile([K0, E], BF16)
                nc.sync.dma_start(out=wa_t[:], in_=wa[g, :, :])
                wp_t = wpool.tile([K0, E], BF16)
                nc.sync.dma_start(out=wp_t[:], in_=wp[g, :, :])
                wa1_t = wpool.tile([128, E], BF16)
                nc.sync.dma_start(out=wa1_t[:], in_=wa1[g, :, :])
                wp1_t = wpool.tile([128, E], BF16)
                nc.sync.dma_start(out=wp1_t[:], in_=wp1[g, :, :])
                wu0_t = wpool.tile([128, E], BF16)
                nc.sync.dma_start(out=wu0_t[:], in_=wu[g, :128, :])
                wu1_t = wpool.tile([128, E], BF16)
                nc.sync.dma_start(out=wu1_t[:], in_=wu[g, 128:, :])
                w2_t = wpool.tile([128, 2 * 128], BF16)
                nc.sync.dma_start(out=w2_t[:], in_=w2[g, :, :])
                cv_t = wpool.tile([128, 2], F32)
                nc.sync.dma_start(out=cv_t[:], in_=cv[g, :, :])
                ch_t = wpool.tile([128, 2], F32)
                nc.sync.dma_start(out=ch_t[:], in_=ch[g, :, :])

                def emit_q(hts_prev, bt_prev):
                    c0p, c1p = bt_prev * BT, (bt_prev + 1) * BT
                    pq = pqpool.tile([128, BT], F32)
                    nc.tensor.matmul(pq[:], w2_t[:, 0:128], hts_prev[0][:],
                                     start=True, stop=False)
                    nc.tensor.matmul(pq[:], w2_t[:, 128:256], hts_prev[1][:],
                                     start=False, stop=True)
                    if ABLATE != "mm_only":
                        # q lives on PSUM partition 0 (w2 replicated over M);
                        # ACT copies it out (Pool can't read PSUM on hw, and
                        # DVE is busy with h evictions; copy shares the relu
                        # act table so no table reload)
                        qt = qpool.tile([1, BT], F32)
                        nc.scalar.activation(qt[:], pq[0:1, :], COPY)
                        nc.sync.dma_start(out=y[g, c0p:c1p], in_=qt[:])

                prev_hts = None
                for bt in range(NBT):
                    c0, c1 = bt * BT, (bt + 1) * BT
                    xt0 = xpool.tile([K0, BT], BF16)
                    nc.sync.dma_start(out=xt0[:], in_=xk0[g, :, c0:c1])
                    xt1 = xt1s[bt % 4]
                    nc.sync.dma_start(out=xt1[0:A, :], in_=xk1[g, :, c0:c1])

                    vts = []
                    for mh in range(2):
                        ms = slice(mh * 128, (mh + 1) * 128)
                        pv = pvpool.tile([128, BT], F32)
                        nc.tensor.matmul(pv[:], wa_t[:, ms], xt0[:],
                                         start=True, stop=False)
                        nc.tensor.matmul(pv[:], wa1_t[:, ms], xt1[:],
                                         start=False, stop=True)
                        if ABLATE != "mm_only":
                            vt = vpool.tile([128, BT], BF16)
                            nc.scalar.activation(vt[:], pv[:], RELU,
                                                 bias=cv_t[:, mh:mh + 1])
                            vts.append(vt)

                    # previous tile's q-stage fills PE time while this tile's
                    # evictions drain
                    if prev_hts is not None:
                        emit_q(prev_hts, bt - 1)

                    rhs0 = vts[0] if ABLATE != "mm_only" else xt0
                    rhs1 = vts[1] if ABLATE != "mm_only" else xt0
                    hts = []
                    for mh in range(2):
                        ms = slice(mh * 128, (mh + 1) * 128)
                        ph = phpool.tile([128, BT], F32)
                        nc.tensor.matmul(ph[:], wp1_t[:, ms], xt1[:],
                                         start=True, stop=False)
                        nc.tensor.matmul(ph[:], wp_t[:, ms], xt0[:],
                                         start=False, stop=False)
                        nc.tensor.matmul(ph[:], wu0_t[:, ms], rhs0[:],
                                         start=False, stop=False)
                        nc.tensor.matmul(ph[:], wu1_t[:, ms], rhs1[:],
                                         start=False, stop=True)
                        if ABLATE != "mm_only":
                            ht = hpool.tile([128, BT], BF16)
                            nc.vector.tensor_scalar(ht[:], ph[:],
                                                    ch_t[:, mh:mh + 1], 0.0,
                                                    op0=ADD, op1=MAX)
                            hts.append(ht)
                    prev_hts = hts if ABLATE != "mm_only" else [xt0, xt0]

                if prev_hts is not None:
                    emit_q(prev_hts, NBT - 1)

    nc.finalize()
    _BUILT[key] = nc
    return nc


def _repeat_loop(tc, repeats):
    # benchmarking aid: run the whole body `repeats` times so wall-clock
    # differences isolate on-device execution time
    from contextlib import nullcontext
    return tc.For_i(0, repeats, 1) if repeats > 1 else nullcontext()


def _prep_in_maps(state, action, g_weight, g_bias, W_Q, W_K, W_V,
                  gx_weight_1, gx_bias_1, gx_weight_2, gx_bias_2):
    f8 = np.float64
    WVt = W_V.T.astype(f8)
    in_maps = []
    for c in range(NCORES):
        xk0 = np.empty((G, K0, B), NPBF)
        xk1 = np.empty((G, A, B), NPBF)
        wa = np.empty((G, K0, E), NPBF)
        wp = np.empty((G, K0, E), NPBF)
        wa1 = np.zeros((G, 128, E), NPBF)
        wp1 = np.zeros((G, 128, E), NPBF)
        wu = np.empty((G, E, E), NPBF)
        w2 = np.empty((G, 128, 2 * 128), NPBF)
        cv = np.empty((G, 128, 2), np.float32)
        ch = np.empty((G, 128, 2), np.float32)
        for g in range(G):
            n = c * G + g
            xk0[g] = state[:, n, :].T
            xk1[g] = action[:, n, :].T
            Gn = g_weight[n].astype(f8)
            U1 = gx_weight_1[n][:E].astype(f8)
            An = (Gn @ WVt).astype(np.float32)
            Pn = (Gn @ U1).astype(np.float32)
            wa[g] = An[:K0]
            wp[g] = Pn[:K0]
            wa1[g, :A] = An[K0:]
            wp1[g, :A] = Pn[K0:]
            wu[g] = gx_weight_1[n][E:]
            w2[g, :, 0:128] = gx_weight_2[n][:128, 0:1]
            w2[g, :, 128:256] = gx_weight_2[n][128:, 0:1]
            cvn = (g_bias[n].astype(f8) @ WVt).astype(np.float32)
            chn = (g_bias[n].astype(f8) @ U1
                   + gx_bias_1[n].astype(f8)).astype(np.float32)
            cv[g, :, 0] = cvn[:128]
            cv[g, :, 1] = cvn[128:]
            ch[g, :, 0] = chn[:128]
            ch[g, :, 1] = chn[128:]
        in_maps.append({
            "xk0": np.ascontiguousarray(xk0),
            "xk1": np.ascontiguousarray(xk1),
            "wa": np.ascontiguousarray(wa),
            "wp": np.ascontiguousarray(wp),
            "wa1": np.ascontiguousarray(wa1),
            "wp1": np.ascontiguousarray(wp1),
            "wu": np.ascontiguousarray(wu),
            "w2": np.ascontiguousarray(w2),
            "cv": np.ascontiguousarray(cv),
            "ch": np.ascontiguousarray(ch),
        })
    return in_maps


def _run(in_maps, repeats=1, **kwargs):
    nc = _build(repeats)
    return run_bass_kernel_spmd(nc, in_maps, list(range(NCORES)), **kwargs)


def _gather(results, gx_bias_2):
    out = np.empty((B, N, 1), np.float32)
    for c in range(NCORES):
        yc = results[c]["y"]
        for g in range(G):
            n = c * G + g
            out[:, n, 0] = yc[g] + gx_bias_2[n, 0]
    return out


def kernel(**inputs):
    inputs = {k: np.asarray(v) for k, v in inputs.items()}
    in_maps = _prep_in_maps(**inputs)
    res = _run(in_maps)
    return _gather(res.results, inputs["gx_bias_2"])
